# revision 1
# baseline (speedup 1.0000x reference)
"""Equivariant attention (gnn_message_passing) on 8 Trainium2 NeuronCores.

Strategy (head-sharded tensor parallel, core c owns head c):

The reference materializes [H, N, E] scores/attn over E=8192 edges. Here the
edge dimension is collapsed onto the N=512 atoms at projection level:

  scores[h, n, e]   = sf[h, n, a_e] + bias[h, edge_map[e]]     (a_e = atom_index)
  attn-softmax per (batch-segment, n) then  out = attn @ vh_edges

factors exactly into dense [N, N] algebra with two tiny per-(segment, atom)
tables (NSEG=16 x N=512):

  D[g, m] = sum_{e in seg g, a_e = m} env_e   * exp(b_e)
  C[g, m] = sum_{e in seg g, a_e = m} env_e^2 * exp(b_e)
  den[g, n]  = sum_m exp(sf[m, n]) * D[g, m]           (one matmul)
  Aagg[m, n] = exp(sf[m, n]) * sum_g C[g, m] / den[g, n]
  out[n, f]  = Aagg^T @ vh[m, f]                        (one matmul)

The running-max subtraction in the reference softmax cancels exactly (up to a
+1e-16 epsilon whose relative effect is ~1e-16) and |scale*sf + b| < 20, so
unnormalized exp is safe in f32.

D/C are built on-device from "slot tensors": host packs per-(atom, segment)
edge lists into a fixed-width [128, 4*16*L2] layout (env & bias values; pads
have env=0 so they vanish), and a single free-axis reduce per table produces
it. Only integer index bookkeeping and value re-layout happen on host.

q/k/v arrive HOST-PRE-TRANSPOSED as qT/kT/vT [CIN, S*N] (channel-major), so
the kernel needs no on-device input transposes: projections read qT slices
directly.  DMAs are issued in first-use order so the PE starts ~1.5us in.

Phase 2 (per-core 64-atom slice): LN + output projection in yT [ci, (s, n)]
orientation.  The per-atom LN scaling is fused into the PE transpose by
replacing the identity operand with diag(scale) (the mean subtraction rides
along as a rank-1 ones x (-mu*rstd) matmul); gamma is folded into the output
weights on device, beta/bo become per-partition biases of the PSUM extract.
Host un-transposes the yT output for free.

Both phases are bf16 on the big operands (inputs, weights, intermediates
bound for matmuls); f32 is kept for the softmax denominators, LN statistics
and the final output.  HW-verified rel err ~8e-3 (gate 2e-2).
"""

import os
import numpy as np

import concourse.bass as bass
import concourse.tile as tile
from concourse import mybir
from concourse.bass_utils import run_bass_kernel_spmd
from concourse.masks import make_identity

# ---------------------------------------------------------------- constants
H, LMAX, NSEG = 8, 2, 16
S = (LMAX + 1) ** 2          # 9 spherical components
N, E, CIN, CH = 512, 8192, 128, 256
D = CH // H                  # 32 per-head channels
F = S * D                    # 288 per-head feature width
NT = N // 128                # 4 atom tiles
NR = N // H                  # 64 atoms per core in the LN/out stage
EPS = 1e-7
SCALE = float(np.sqrt(D / 3.0) / D)
L_OF_M = np.floor(np.sqrt(np.arange(S))).astype(np.int64)
F32 = mybir.dt.float32
F32R = mybir.dt.float32r
BF16 = mybir.dt.bfloat16
AF = mybir.ActivationFunctionType
ALU = mybir.AluOpType

import ml_dtypes
NP_BF16 = ml_dtypes.bfloat16

_DBG = bool(int(os.environ.get("KBDBG", "0")))


def _split_multiwaits(nc: bass.Bass, limit: int = 1):
    """This walrus build rejects instructions carrying more than one semaphore
    wait (and Drains carrying any). Hoist excess waits onto NOPs inserted just
    before the instruction on the same engine - semantically identical."""
    for f in nc.m.functions:
        for blk in f.blocks:
            changed = False
            out = []
            for inst in blk.instructions:
                si = inst.sync_info
                waits = list(si.on_wait) if si is not None else []
                keep = 0 if inst.opcode == "Drain" else limit
                if len(waits) > keep:
                    hoist = waits[: len(waits) - keep]
                    rest = waits[len(waits) - keep:]
                    for w in hoist:
                        nop = mybir.InstNoOp(
                            name=f"{inst.name}-w{len(out)}", ins=[], outs=[]
                        )
                        nop.engine = inst.engine
                        nop.sync_info = mybir.SyncInfo(on_wait=[w], on_update=[])
                        out.append(nop)
                    inst.sync_info = mybir.SyncInfo(
                        on_wait=rest, on_update=list(si.on_update)
                    )
                    changed = True
                out.append(inst)
            if changed:
                blk.instructions = out


def build_bass(L2: int, loop_R: int | None = None) -> bass.Bass:
    """One SPMD program; per-core data (weight slices, bias slots) comes in as
    inputs. L2 = slot width per (atom, segment) cell."""
    W = NT * NSEG * L2  # slot tensor free width per partition

    nc = bass.Bass("TRN2", target_bir_lowering=False, debug=False, num_devices=H)

    # ------------------------------------------------------------- tensors
    # host-pre-transposed bf16 inputs: qT/kT [i, (s, m)]; vT mt-major
    # [i, (t, s, j)] so each quarter is a contiguous DMA
    qT_d = nc.dram_tensor("qT", [CIN, S * N], BF16, kind="ExternalInput")
    kT_d = nc.dram_tensor("kT", [CIN, S * N], BF16, kind="ExternalInput")
    vT_d = nc.dram_tensor("vT", [CIN, S * N], BF16, kind="ExternalInput")
    wq_d = nc.dram_tensor("wq", [CIN, S * D], BF16, kind="ExternalInput")  # [i,(s,o)]
    wk_d = nc.dram_tensor("wk", [CIN, S * D], BF16, kind="ExternalInput")
    wv_d = nc.dram_tensor("wv", [CIN, S * D], BF16, kind="ExternalInput")
    bqkv_d = nc.dram_tensor("bqkv", [D, 3], F32, kind="ExternalInput")
    bvrow_d = nc.dram_tensor("bvrow", [1, D], F32, kind="ExternalInput")
    envs_d = nc.dram_tensor("envs", [128, W], BF16, kind="ExternalInput")
    bs_d = nc.dram_tensor("bs", [128, W], BF16, kind="ExternalInput")
    ao_d = nc.dram_tensor("ao", [N, F], BF16, kind="ExternalOutput")

    with tile.TileContext(nc) as tc:
        with (
            tc.tile_pool(name="const", bufs=1) as cpool,
            tc.tile_pool(name="feat", bufs=1) as featp,
            tc.tile_pool(name="work", bufs=1) as workp,
            tc.tile_pool(name="aop", bufs=1) as aop,
            tc.tile_pool(name="ps", bufs=4, space="PSUM") as psp,
            tc.tile_pool(name="pop", bufs=1, space="PSUM") as pop,
        ):
            def ps_tile(shape):
                return psp.tile(shape, F32, tag="ps", name="ps")

            # -------- DMA issue, first-use order, spread over engines ------
            # sync queue:   wq, qT thirds, vT quarters (mt-major layout)
            # scalar queue: wk, wv, kT thirds, envs, bs
            # gpsimd queue: tiny constants
            TH = 3 * N
            wq_sb = cpool.tile([CIN, S * D], BF16, tag="wq", name="wq")
            wk_sb = cpool.tile([CIN, S * D], BF16, tag="wk", name="wk")
            wv_sb = cpool.tile([CIN, S * D], BF16, tag="wv", name="wv")
            qTc = [cpool.tile([CIN, TH], BF16, tag=f"qT{t}", name=f"qT{t}")
                   for t in range(3)]
            kTc = [cpool.tile([CIN, TH], BF16, tag=f"kT{t}", name=f"kT{t}")
                   for t in range(3)]
            # vT quarter mt holds [i, (s, m_local)] for atom tile mt
            vTq = [cpool.tile([CIN, S * 128], BF16, tag=f"vT{m}", name=f"vT{m}")
                   for m in range(NT)]
            envs_sb = cpool.tile([128, W], BF16, tag="envs", name="envs")
            bs_sb = cpool.tile([128, W], BF16, tag="bs", name="bs")

            def vtq_src(m):
                return vT_d[:].rearrange("i (t x) -> i t x", t=NT)[:, m, :]

            nc.sync.dma_start(wq_sb[:], wq_d[:])
            for t in range(3):
                nc.sync.dma_start(qTc[t][:], qT_d[:, t * TH:(t + 1) * TH])
            nc.sync.dma_start(vTq[0][:], vtq_src(0))
            nc.sync.dma_start(vTq[1][:], vtq_src(1))
            nc.scalar.dma_start(wk_sb[:], wk_d[:])
            for t in range(3):
                nc.scalar.dma_start(kTc[t][:], kT_d[:, t * TH:(t + 1) * TH])
            nc.scalar.dma_start(wv_sb[:], wv_d[:])
            nc.gpsimd.dma_start(envs_sb[:], envs_d[:])
            nc.gpsimd.dma_start(bs_sb[:], bs_d[:])
            nc.scalar.dma_start(vTq[2][:], vtq_src(2))
            nc.scalar.dma_start(vTq[3][:], vtq_src(3))
            bqkv_sb = cpool.tile([D, 3], F32, tag="bqkv", name="bqkv")
            nc.gpsimd.dma_start(bqkv_sb[:], bqkv_d[:])
            bvrow_sb = cpool.tile([128, D], F32, tag="bvrow", name="bvrow")
            nc.gpsimd.dma_start(bvrow_sb[:], bvrow_d[0:1, :].to_broadcast([128, D]))
            ident = cpool.tile([128, 128], F32, tag="ident", name="ident")
            make_identity(nc, ident[:])
            eps16 = cpool.tile([1, NSEG], F32, tag="eps16", name="eps16")
            nc.gpsimd.memset(eps16[:], 1e-16)
            ones_n = cpool.tile([1, N], F32, tag="ones_n", name="ones_n")
            nc.gpsimd.memset(ones_n[:], 1.0)

            import contextlib as _ctl
            _loop = tc.For_i(0, loop_R, 1) if loop_R else _ctl.nullcontext()
            with _loop:
                copy_engines = [nc.scalar, nc.vector]
                cp_i = 0

                def copy_alt(dst_ap, src_ap):
                    nonlocal cp_i
                    eng = copy_engines[cp_i % 2]
                    cp_i += 1
                    if eng is nc.scalar:
                        eng.copy(dst_ap, src_ap)
                    else:
                        eng.tensor_copy(out=dst_ap, in_=src_ap)

                def copy_dve(dst_ap, src_ap, eng=None):
                    if eng is nc.scalar:
                        eng.copy(dst_ap, src_ap)
                    else:
                        nc.vector.tensor_copy(out=dst_ap, in_=src_ap)

                # ------------------------------ D / C tables (early, off-path)
                ebs = workp.tile([128, W], F32, tag="ebs", name="ebs")
                nc.scalar.activation(ebs[:], bs_sb[:], AF.Exp)
                wD = workp.tile([128, W], F32, tag="wD", name="wD")
                nc.vector.tensor_tensor(out=wD[:], in0=envs_sb[:], in1=ebs[:], op=ALU.mult)
                wC = workp.tile([128, W], F32, tag="wC", name="wC")
                nc.vector.tensor_tensor(out=wC[:], in0=wD[:], in1=envs_sb[:], op=ALU.mult)
                d_t = featp.tile([128, NT * NSEG], F32, tag="d_t", name="d_t")  # [m_p, (mt, g)]
                c_t = featp.tile([128, NT * NSEG], F32, tag="c_t", name="c_t")
                with nc.allow_low_precision(reason="f32r is 32-bit storage"):
                    nc.vector.reduce_sum(
                        out=d_t[:].rearrange("p (t g) -> p t g", t=NT).bitcast(F32R),
                        in_=wD[:].rearrange("p (t g j) -> p t g j", t=NT, g=NSEG),
                        axis=mybir.AxisListType.X,
                    )
                nc.vector.reduce_sum(
                    out=c_t[:].rearrange("p (t g) -> p t g", t=NT),
                    in_=wC[:].rearrange("p (t g j) -> p t g j", t=NT, g=NSEG),
                    axis=mybir.AxisListType.X,
                )
                # ------- fq / fk chunk-streamed projection + psf accumulation
                # chunk layout: rows (s_local*32+o), chunks s=0..2 / 3..5 / 6..8
                # (96 rows per chunk so matmul outs land at base 0/32/64);
                # psf[mt] accumulates across chunks in 4 held banks (tags
                # shared with the po accumulators, which start strictly later)
                fq = [featp.tile([96, N], BF16, tag=f"fq{c}", name=f"fq{c}")
                      for c in range(3)]
                fk = [featp.tile([96, N], BF16, tag=f"fk{c}", name=f"fk{c}")
                      for c in range(3)]
                psf = [pop.tile([128, N], F32, tag=f"acc{mt}", name=f"psf{mt}")
                       for mt in range(NT)]
                # within chunk 0 the s components sit in row order (1, 2, 0)
                # so the biased s=0 rows are 64-aligned for the PSUM read
                # (fk uses the same permutation, so scores are unchanged)
                ROWOF = {0: 2, 1: 0, 2: 1}
                for chunk in range(3):
                    for t_c, w_sb, f_dst, t_idx in ((qTc, wq_sb, fq, 0),
                                                    (kTc, wk_sb, fk, 1)):
                        pp = ps_tile([96, N])
                        for j in range(3):
                            s = chunk * 3 + j
                            r = ROWOF[j] if chunk == 0 else j
                            nc.tensor.matmul(
                                pp[r * D:(r + 1) * D, :],
                                lhsT=w_sb[:, s * D:(s + 1) * D],
                                rhs=t_c[chunk][:, j * N:(j + 1) * N],
                                start=True, stop=True,
                            )
                        ceng = nc.scalar if t_idx == 0 else nc.vector
                        if chunk == 0:
                            # bias on s=0 rows (l=0 invariant component)
                            copy_dve(f_dst[0][0:64, :], pp[0:64, :], ceng)
                            nc.vector.tensor_scalar_add(
                                f_dst[0][64:96, :], pp[64:96, :],
                                bqkv_sb[:, t_idx:t_idx + 1])
                        else:
                            copy_dve(f_dst[chunk][:], pp[:], ceng)
                    for mt in range(NT):
                        nc.tensor.matmul(
                            psf[mt][:],
                            lhsT=fk[chunk][:, mt * 128:(mt + 1) * 128],
                            rhs=fq[chunk][:],
                            start=(chunk == 0), stop=(chunk == 2),
                            skip_group_check=True,
                        )

                # C transposed to [g, m]
                c_sb = featp.tile([NSEG, N], F32, tag="c_sb", name="c_sb")
                for mt in range(NT):
                    pc = ps_tile([NSEG, 128])
                    nc.tensor.transpose(
                        pc[:], c_t[:, mt * NSEG:(mt + 1) * NSEG], ident[:]
                    )
                    nc.vector.tensor_copy(out=c_sb[:, mt * 128:(mt + 1) * 128].bitcast(F32R), in_=pc[:])

                # -------------------------------- vhn [m, (s,o)] per m-tile
                vhn = [featp.tile([128, F], BF16, tag=f"vhn{mt}", name=f"vhn{mt}") for mt in range(NT)]
                for mt in range(NT):
                    pv = ps_tile([128, F])
                    for s in range(S):
                        nc.tensor.matmul(
                            pv[:, s * D:(s + 1) * D],
                            lhsT=vTq[mt][:, s * 128:(s + 1) * 128],
                            rhs=wv_sb[:, s * D:(s + 1) * D],
                            start=True, stop=True,
                        )
                    nc.vector.tensor_copy(out=vhn[mt][:, D:F], in_=pv[:, D:F])
                    nc.vector.tensor_tensor(
                        out=vhn[mt][:, 0:D],
                        in0=pv[:, 0:D], in1=bvrow_sb[:], op=ALU.add,
                    )

                # ------- exp + denominator, split in n-halves (h-major) so
                # the dd -> aggt -> att-out chain pipelines per half
                HN = N // 2
                exp_sf = [featp.tile([128, N], F32, tag=f"esf{mt}",
                                     name=f"esf{mt}") for mt in range(NT)]
                pden = [ps_tile([NSEG, HN]) for h in range(2)]
                for h in range(2):
                    nc.tensor.matmul(
                        pden[h][:], lhsT=eps16[:].bitcast(F32R),
                        rhs=ones_n[:, 0:HN].bitcast(F32R), start=True, stop=False,
                        skip_group_check=True,
                    )
                for mt in range(NT):
                    nc.scalar.activation(exp_sf[mt][:].bitcast(F32R), psf[mt][:],
                                         AF.Exp, scale=SCALE)
                    for h in range(2):
                        nc.tensor.matmul(
                            pden[h][:],
                            lhsT=d_t[:, mt * NSEG:(mt + 1) * NSEG].bitcast(F32R),
                            rhs=exp_sf[mt][:, h * HN:(h + 1) * HN].bitcast(F32R),
                            start=False, stop=(mt == NT - 1),
                            skip_group_check=True,
                        )

                # ------- per half: dd; per m-tile: aggt mult; att-out
                aggt = [[featp.tile([128, HN], BF16, tag=f"agg{mt}h{h}",
                                    name=f"agg{mt}h{h}") for h in range(2)]
                        for mt in range(NT)]
                po = [pop.tile([128, F], F32, tag=f"acc{nt}", name=f"po{nt}")
                      for nt in range(NT)]
                dd = [featp.tile([NSEG, HN], F32, tag=f"ddh{h}", name=f"ddh{h}")
                      for h in range(2)]
                for h in range(2):
                    with nc.allow_low_precision(reason="f32r is 32-bit storage"):
                        nc.vector.reciprocal(dd[h][:].bitcast(F32R), pden[h][:])
                    for mt in range(NT):
                        pT = ps_tile([128, HN])
                        nc.tensor.matmul(
                            pT[:],
                            lhsT=c_sb[:, mt * 128:(mt + 1) * 128].bitcast(F32R),
                            rhs=dd[h][:].bitcast(F32R),
                            start=True, stop=True,
                        )
                        nc.vector.tensor_tensor(
                            out=aggt[mt][h][:],
                            in0=exp_sf[mt][:, h * HN:(h + 1) * HN],
                            in1=pT[:], op=ALU.mult)
                        for j in range(2):
                            nt = 2 * h + j
                            nc.tensor.matmul(
                                po[nt][:],
                                lhsT=aggt[mt][h][:, j * 128:(j + 1) * 128],
                                rhs=vhn[mt][:],
                                start=(mt == 0), stop=(mt == NT - 1),
                                skip_group_check=True,
                            )
                # two ao DMAs; slice the t axis AFTER the rearrange (the
                # slice-then-rearrange form writes wrong DRAM locations)
                for pair in range(2):
                    ao = aop.tile([128, 2 * F], BF16, tag=f"ao{pair}",
                                  name=f"ao{pair}")
                    for j in range(2):
                        copy_alt(ao[:, j * F:(j + 1) * F], po[2 * pair + j][:])
                    nc.sync.dma_start(
                        ao_d[:].rearrange("(t p) f -> p t f", t=NT)
                        [:, 2 * pair:2 * pair + 2, :],
                        ao[:].rearrange("p (t f) -> p t f", t=2))

    _split_multiwaits(nc)
    return nc


def build_phase2(loop_R: int | None = None) -> bass.Bass:
    """Equivariant layernorm + output projection on a 64-atom slice.
    Input lnin [64, (s, ch)]; output yT [ci, (s, n)] (host un-transposes).
    gamma/beta are folded into the post-transpose PSUM->SBUF copies as
    per-partition tensor_scalar ops; same NEFF on all cores."""
    nc = bass.Bass("TRN2", target_bir_lowering=False, debug=False, num_devices=H)
    lnin_d = nc.dram_tensor("lnin", [NR, S * CH], BF16, kind="ExternalInput")
    gcol_d = nc.dram_tensor("gcol", [128, 2 * (LMAX + 1)], F32, kind="ExternalInput")
    bcol_d = nc.dram_tensor("bcol", [128, 2], BF16, kind="ExternalInput")
    # compact per-l output weights: [c_half, i, (l, ci)]
    woe_d = nc.dram_tensor("woe", [2, 128, (LMAX + 1) * CIN], BF16, kind="ExternalInput")
    bo_d = nc.dram_tensor("bo", [CIN, 1], F32, kind="ExternalInput")
    y_d = nc.dram_tensor("yT", [CIN, S * NR], F32, kind="ExternalOutput")

    with tile.TileContext(nc) as tc:
        with (
            tc.tile_pool(name="const", bufs=1) as cpool,
            tc.tile_pool(name="work", bufs=1) as workp,
            tc.tile_pool(name="tp", bufs=4) as tpp,
            tc.tile_pool(name="ps", bufs=1, space="PSUM") as psp,
            tc.tile_pool(name="plbp", bufs=4, space="PSUM") as plbp,
            tc.tile_pool(name="pyg", bufs=3, space="PSUM") as pygp,
        ):
            def ps_tile(shape):
                return psp.tile(shape, F32, tag="ps", name="ps")

            lnin = workp.tile([NR, S * CH], BF16, tag="lnin", name="lnin")
            # section DMAs so the l=0 chain starts early
            nc.sync.dma_start(lnin[:, 0:CH], lnin_d[:, 0:CH])
            nc.sync.dma_start(lnin[:, 4 * CH:S * CH], lnin_d[:, 4 * CH:S * CH])
            nc.sync.dma_start(lnin[:, CH:4 * CH], lnin_d[:, CH:4 * CH])
            woe_sb = [
                cpool.tile([128, (LMAX + 1) * CIN], BF16, tag=f"woe{c}", name=f"woe{c}")
                for c in range(2)
            ]
            gcol_sb = cpool.tile([128, 2 * (LMAX + 1)], F32, tag="gcol", name="gcol")
            bcol_sb = cpool.tile([128, 2], BF16, tag="bcol", name="bcol")
            bo_sb = cpool.tile([CIN, 1], F32, tag="bo", name="bo")
            for c in range(2):
                nc.sync.dma_start(woe_sb[c][:], woe_d[c, :, :])
            nc.sync.dma_start(gcol_sb[:], gcol_d[:])
            nc.sync.dma_start(bcol_sb[:], bcol_d[:])
            nc.sync.dma_start(bo_sb[:], bo_d[:])
            ident = cpool.tile([128, 128], BF16, tag="ident", name="ident")
            make_identity(nc, ident[:])
            eps_sb = cpool.tile([128, 1], F32, tag="epsc", name="epsc")
            nc.gpsimd.memset(eps_sb[:], EPS)

            ones_r = cpool.tile([NR, 128], BF16, tag="ones_r", name="ones_r")
            nc.gpsimd.memset(ones_r[:], 1.0)

            import contextlib as _ctl
            _loop = tc.For_i(0, loop_R, 1) if loop_R else _ctl.nullcontext()
            with _loop:
                # ---- LN statistics (per-atom scalars)
                # l=0: one-pass LN over CH: var = E[x^2] - mu^2
                x0 = lnin[:, 0:CH]
                sc0 = workp.tile([NR, CH], F32, tag="sc0", name="sc0")
                mu = workp.tile([NR, 1], F32, tag="mu", name="mu")
                nc.scalar.activation(sc0[:], x0, AF.Copy, scale=1.0 / CH,
                                     accum_out=mu[:])
                sq0 = workp.tile([NR, CH], F32, tag="sq0", name="sq0")
                vs = workp.tile([NR, 1], F32, tag="vs", name="vs")
                nc.vector.tensor_tensor(out=sq0[:], in0=x0, in1=x0, op=ALU.mult)
                nc.vector.reduce_sum(out=vs[:], in_=sq0[:],
                                     axis=mybir.AxisListType.X)
                mu2 = workp.tile([NR, 1], F32, tag="mu2", name="mu2")
                nc.gpsimd.tensor_tensor(out=mu2[:], in0=mu[:], in1=mu[:], op=ALU.mult)
                ebias = workp.tile([NR, 1], F32, tag="ebias", name="ebias")
                nc.gpsimd.tensor_scalar(out=ebias[:], in0=mu2[:], scalar1=-1.0,
                                        scalar2=EPS, op0=ALU.mult, op1=ALU.add)
                sd = workp.tile([NR, 1], F32, tag="sd", name="sd")
                nc.scalar.activation(sd[:], vs[:], AF.Sqrt, scale=1.0 / CH,
                                     bias=ebias[:, 0:1])
                rstd = workp.tile([NR, 1], F32, tag="rstd", name="rstd")
                nc.vector.reciprocal(rstd[:], sd[:])
                # l=2 stats split across Act (Square+accum) and DVE halves;
                # l=1 on Act; recips on DVE
                rr = {}
                lo2, hi2 = 4 * CH, S * CH
                w2 = hi2 - lo2
                mid = lo2 + w2 // 2
                ms2a = workp.tile([NR, 1], F32, tag="ms2a", name="ms2a")
                sq2a = workp.tile([NR, w2 // 2], F32, tag="sq2a", name="sq2a")
                nc.scalar.activation(sq2a[:], lnin[:, lo2:mid], AF.Square,
                                     accum_out=ms2a[:])
                sq2b = workp.tile([NR, w2 // 2], BF16, tag="sq2b", name="sq2b")
                ms2b = workp.tile([NR, 1], F32, tag="ms2b", name="ms2b")
                nc.vector.tensor_tensor(out=sq2b[:], in0=lnin[:, mid:hi2],
                                        in1=lnin[:, mid:hi2], op=ALU.mult)
                with nc.allow_low_precision(reason="rms over 1280 terms"):
                    nc.vector.reduce_sum(out=ms2b[:], in_=sq2b[:],
                                         axis=mybir.AxisListType.X)
                ms2 = workp.tile([NR, 1], F32, tag="ms2", name="ms2")
                nc.gpsimd.tensor_tensor(out=ms2[:], in0=ms2a[:], in1=ms2b[:],
                                        op=ALU.add)
                sdl2 = workp.tile([NR, 1], F32, tag="sd2", name="sd2")
                nc.scalar.activation(sdl2[:], ms2[:], AF.Sqrt, scale=1.0 / w2,
                                     bias=eps_sb[0:NR, 0:1])
                rr2 = workp.tile([NR, 1], F32, tag="rr2", name="rr2")
                nc.vector.reciprocal(rr2[:], sdl2[:])
                rr[2] = rr2
                lo1, hi1 = CH, 4 * CH
                w1 = hi1 - lo1
                ms1 = workp.tile([NR, 1], F32, tag="ms1", name="ms1")
                sq1 = workp.tile([NR, w1], F32, tag="sq1", name="sq1")
                nc.scalar.activation(sq1[:], lnin[:, lo1:hi1], AF.Square,
                                     accum_out=ms1[:])
                sdl1 = workp.tile([NR, 1], F32, tag="sd1", name="sd1")
                nc.scalar.activation(sdl1[:], ms1[:], AF.Sqrt, scale=1.0 / w1,
                                     bias=eps_sb[0:NR, 0:1])
                rr1 = workp.tile([NR, 1], F32, tag="rr1", name="rr1")
                nc.vector.reciprocal(rr1[:], sdl1[:])
                rr[1] = rr1

                # ---- gamma folded into weights (DVE bf16 fast mode)
                woe_g = [workp.tile([128, (LMAX + 1) * CIN], BF16, tag=f"wg{c}",
                                    name=f"wg{c}") for c in range(2)]
                for c in range(2):
                    for l in range(LMAX + 1):
                        nc.vector.tensor_scalar_mul(
                            woe_g[c][:, l * CIN:(l + 1) * CIN],
                            woe_sb[c][:, l * CIN:(l + 1) * CIN],
                            gcol_sb[:, 2 * l + c:2 * l + c + 1])
                # beta contribution: pbw[ci] = sum_ch beta[ch] * Wo[0][ch, ci]
                pbw = ps_tile([CIN, 1])
                for c in range(2):
                    nc.tensor.matmul(
                        pbw[:], lhsT=woe_sb[c][:, 0:CIN],
                        rhs=bcol_sb[:, c:c + 1],
                        start=(c == 0), stop=(c == 1))
                bo0 = workp.tile([CIN, 1], F32, tag="bo0", name="bo0")
                nc.vector.tensor_tensor(out=bo0[:], in0=pbw[:], in1=bo_sb[:],
                                        op=ALU.add)

                mr = workp.tile([NR, 1], F32, tag="mr", name="mr")
                nc.gpsimd.tensor_tensor(out=mr[:], in0=mu[:], in1=rstd[:], op=ALU.mult)
                nmr = workp.tile([NR, 1], F32, tag="nmr", name="nmr")
                nc.gpsimd.tensor_scalar_mul(nmr[:], mr[:], -1.0)

                # ---- diag(scale) tiles: the transpose matmul applies the
                # per-atom LN scaling for free (rhs = diag instead of I)
                diag = {}
                for l, scl in ((2, rr[2]), (0, rstd), (1, rr[1])):
                    dg = workp.tile([NR, NR], BF16, tag=f"diag{l}", name=f"diag{l}")
                    nc.gpsimd.tensor_scalar_mul(dg[:], ident[0:NR, 0:NR],
                                                scl[:, 0:1])
                    diag[l] = dg
                dnm = workp.tile([NR, NR], BF16, tag="dnm", name="dnm")
                nc.gpsimd.tensor_scalar_mul(dnm[:], ident[0:NR, 0:NR], nmr[:, 0:1])

                # ---- per s (l=2 block first): scale+transpose -> lnT -> yT
                y_sb = workp.tile([CIN, S * NR], F32, tag="ysb", name="ysb")
                out_dma = [nc.sync, nc.scalar]
                groups = [(4, 5), (6, 7), (8, 0), (1, 2), (3,)]
                for gi, grp in enumerate(groups):
                    pl = plbp.tile([128, 4 * NR], F32, tag="plb", name="plb")
                    for k, s in enumerate(grp):
                        l = int(L_OF_M[s])
                        for c in range(2):
                            col = (2 * k + c) * NR
                            nc.tensor.matmul(
                                pl[:, col:col + NR],
                                lhsT=lnin[:, s * CH + c * 128: s * CH + (c + 1) * 128],
                                rhs=diag[l][:],
                                start=True, stop=(s > 0),
                                skip_group_check=True,
                            )
                            if s == 0:
                                nc.tensor.matmul(
                                    pl[:, col:col + NR],
                                    lhsT=ones_r[:],
                                    rhs=dnm[:],
                                    start=False, stop=True,
                                    skip_group_check=True,
                                )
                    lnT = tpp.tile([128, 4 * NR], BF16, tag="lnT", name="lnT")
                    nc.scalar.copy(lnT[:, 0:len(grp) * 2 * NR],
                                   pl[:, 0:len(grp) * 2 * NR])
                    for k, s in enumerate(grp):
                        l = int(L_OF_M[s])
                        py = pygp.tile([CIN, NR], F32, tag="pyg", name="pyg")
                        for c in range(2):
                            col = (2 * k + c) * NR
                            nc.tensor.matmul(
                                py[:],
                                lhsT=woe_g[c][:, l * CIN:(l + 1) * CIN],
                                rhs=lnT[:, col:col + NR],
                                start=(c == 0), stop=(c == 1),
                            )
                        bias = bo0[:, 0:1] if s == 0 else bo_sb[:, 0:1]
                        nc.vector.tensor_scalar_add(
                            y_sb[:, s * NR:(s + 1) * NR], py[:], bias)
                    if grp == (8, 0):
                        nc.sync.dma_start(y_d[:, 4 * NR:S * NR],
                                          y_sb[:, 4 * NR:S * NR])
                        nc.scalar.dma_start(y_d[:, 0:NR], y_sb[:, 0:NR])
                    elif grp == (3,):
                        nc.sync.dma_start(y_d[:, NR:4 * NR],
                                          y_sb[:, NR:4 * NR])

    _split_multiwaits(nc)
    return nc


# ------------------------------------------------------------------ host side
def _prep_inputs(inputs: dict[str, np.ndarray]):
    """Split the full inputs into per-core in_maps for the two phases
    (index bookkeeping and value re-layout only; all arithmetic on device)."""
    q = np.asarray(inputs["q"], np.float32).reshape(N, S, CIN)
    k = np.asarray(inputs["k"], np.float32).reshape(N, S, CIN)
    v = np.asarray(inputs["v"], np.float32).reshape(N, S, CIN)
    # host pre-transpose to [i, (s, m)] and cast to bf16; vT mt-major
    qT = np.ascontiguousarray(q.transpose(2, 1, 0).reshape(CIN, S * N)).astype(NP_BF16)
    kT = np.ascontiguousarray(k.transpose(2, 1, 0).reshape(CIN, S * N)).astype(NP_BF16)
    vT = np.ascontiguousarray(
        v.reshape(NT, 128, S, CIN).transpose(3, 0, 2, 1).reshape(CIN, S * N)
    ).astype(NP_BF16)
    env = np.asarray(inputs["envelope"], np.float32)
    attn_bias = np.asarray(inputs["attn_bias"], np.float32)
    a_idx = np.asarray(inputs["atom_index"]).astype(np.int64)
    b_idx = np.asarray(inputs["batch_index"]).astype(np.int64)
    e_map = np.asarray(inputs["edge_map_tab"]).astype(np.int64)
    Wq = np.asarray(inputs["Wq"], np.float32)
    Wk = np.asarray(inputs["Wk"], np.float32)
    Wv = np.asarray(inputs["Wv"], np.float32)
    bq = np.asarray(inputs["bq"], np.float32)
    bk = np.asarray(inputs["bk"], np.float32)
    bv = np.asarray(inputs["bv"], np.float32)
    gamma = np.asarray(inputs["gamma"], np.float32)
    beta = np.asarray(inputs["beta"], np.float32)
    Wo = np.asarray(inputs["Wo"], np.float32)
    bo = np.asarray(inputs["bo"], np.float32)

    # ---- slot layout for the (atom, segment) cells
    cell = a_idx * NSEG + b_idx                      # [E]
    order = np.argsort(cell, kind="stable")
    cell_s = cell[order]
    counts = np.bincount(cell_s, minlength=N * NSEG)
    L2 = int(counts.max())
    starts = np.zeros(N * NSEG, np.int64)
    starts[1:] = np.cumsum(counts)[:-1]
    rank = np.arange(E) - starts[cell_s]             # rank within cell
    m_s = cell_s // NSEG
    g_s = cell_s % NSEG
    p_s = m_s % 128
    t_s = m_s // 128
    col = (t_s * NSEG + g_s) * L2 + rank             # free-dim position
    Wd = NT * NSEG * L2
    env_e = env[e_map]                               # value gather (re-layout)
    envS = np.zeros((128, Wd), NP_BF16)
    envS[p_s, col] = env_e[order]
    bS_all = []
    for h in range(H):
        bs = np.zeros((128, Wd), NP_BF16)
        bs[p_s, col] = attn_bias[h, e_map][order]
        bS_all.append(bs)

    # ---- per-head weight slices, expanded per spherical component, [i,(s,o)]
    WqE = Wq[L_OF_M]                                 # [9, CIN, CH]
    WkE = Wk[L_OF_M]
    WvE = Wv[L_OF_M]

    in_maps1 = []
    for h in range(H):
        sl = slice(h * D, (h + 1) * D)
        in_maps1.append({
            "qT": qT, "kT": kT, "vT": vT,
            "wq": np.ascontiguousarray(
                WqE[:, :, sl].transpose(1, 0, 2).reshape(CIN, S * D)).astype(NP_BF16),
            "wk": np.ascontiguousarray(
                WkE[:, :, sl].transpose(1, 0, 2).reshape(CIN, S * D)).astype(NP_BF16),
            "wv": np.ascontiguousarray(
                WvE[:, :, sl].transpose(1, 0, 2).reshape(CIN, S * D)).astype(NP_BF16),
            "bqkv": np.ascontiguousarray(
                np.stack([bq[sl], bk[sl], bv[sl]], axis=1)
            ),
            "bvrow": np.ascontiguousarray(bv[sl].reshape(1, D)),
            "envs": envS,
            "bs": bS_all[h],
        })

    # ---- phase-2 constants
    # gcol[p, 2l+c] = gamma[l, c*128+p];  bcol[p, c] = beta[c*128+p]
    gcol = np.zeros((128, 2 * (LMAX + 1)), np.float32)
    for l in range(LMAX + 1):
        for c in range(2):
            gcol[:, 2 * l + c] = gamma[l, c * 128:(c + 1) * 128]
    bcol = np.stack([beta[0:128], beta[128:256]], axis=1).astype(NP_BF16)
    woe = np.zeros((2, 128, (LMAX + 1) * CIN), NP_BF16)
    for c in range(2):
        woe[c] = Wo[:, c * 128:(c + 1) * 128, :].transpose(1, 0, 2).reshape(
            128, (LMAX + 1) * CIN).astype(NP_BF16)
    p2_const = {"gcol": gcol, "bcol": bcol, "woe": woe,
                "bo": np.ascontiguousarray(bo.reshape(CIN, 1))}
    return in_maps1, L2, p2_const


def _reorder_ao(ao_all: list[np.ndarray]) -> list[np.ndarray]:
    """[h][N, (s,d)] -> per-core [64, (s, h*D+d)] slices (pure data movement)."""
    full = np.stack([np.asarray(a).reshape(N, S, D) for a in ao_all], axis=2)
    full = full.reshape(N, S * CH)                                # [N, (S, H*D)]
    return [np.ascontiguousarray(full[c * NR:(c + 1) * NR]).astype(NP_BF16)
            for c in range(H)]


_BUILD_CACHE: dict = {}


def kernel(**inputs) -> np.ndarray:
    in_maps1, L2, p2_const = _prep_inputs(inputs)
    nc1 = _BUILD_CACHE.get(("p1", L2))
    if nc1 is None:
        nc1 = build_bass(L2)
        _BUILD_CACHE[("p1", L2)] = nc1
    res1 = run_bass_kernel_spmd(nc1, in_maps1, core_ids=list(range(H)))
    lnin_slices = _reorder_ao([r["ao"] for r in res1.results])

    nc2 = _BUILD_CACHE.get("p2")
    if nc2 is None:
        nc2 = build_phase2()
        _BUILD_CACHE["p2"] = nc2
    in_maps2 = [{"lnin": lnin_slices[c], **p2_const} for c in range(H)]
    res2 = run_bass_kernel_spmd(nc2, in_maps2, core_ids=list(range(H)))
    # yT [ci, (s, n_local)] per core -> y [N, S, CIN]
    y = np.zeros((N, S, CIN), np.float32)
    for c in range(H):
        yt = res2.results[c]["yT"].reshape(CIN, S, NR)
        y[c * NR:(c + 1) * NR] = yt.transpose(2, 1, 0)
    return np.ascontiguousarray(y)

